# revision 27
# baseline (speedup 1.0000x reference)
"""Trainium2 Bass kernel for the spiking CapsNet forward pass (nn_CapsNet).

v2 strategy (8 NeuronCores):
  Phase A (batch-parallel, 4 images/core):
    conv1 once in f32r, conv-layer membrane dynamics for 5 steps
    (kc0 spike chain prioritized so the PE can start early), then the
    prim conv kc-major in two weight passes over all 5 timesteps.
  AllToAll: prim spikes as uint8, batch-shard -> route-shard (184KB).
  Phase B (route-parallel, 144 routes/core, full batch):
    u_hat on PE in fp16 with the conv bias fused into the PSUM
    evacuation (DVE/Act alternating); digit-caps loop with the
    membrane chain (a-pass / spikes / reset / trace) scheduled to run
    during each step's AllGather: spikes on the Activation engine
    (Sign+Relu), a-pass on Pool before the collective, reset/trace as
    slack work after the routing chain.  s_j partials are exchanged
    with a 5120-float AllGather at t=0..3 only; out_mem accumulates
    LOCAL route partials each step and the host sums the 8 cores.
Host: im2col + weight relayouts, final classes reduction.
"""

import numpy as np

N_CORES = 8
T = 5
B = 32
BL = B // N_CORES          # local batch (4)
R = 1152
RL = R // N_CORES          # local routes (144)
CO = 160                   # (o,c) pairs, ordered co = o*10 + c
DECAY_TR = np.float32(np.exp(np.float32(-1.0 / 1.5)))
ALPHA = np.float32(np.float32(0.0008) / np.float32(32.0))

_CACHE = {}


def _build_program(reps=1, stage="full", solo=False):
    import concourse.bass as bass
    import concourse.mybir as mybir
    import concourse.tile as tile
    from concourse import bacc

    f32 = mybir.dt.float32
    f32r = mybir.dt.float32r
    f16 = mybir.dt.float16
    u8 = mybir.dt.uint8

    nc = bacc.Bacc("TRN2", target_bir_lowering=False, debug=False,
                   num_devices=N_CORES)

    # ---- external I/O (per-core values supplied by host) ----
    im2_d = nc.dram_tensor("im2", [81, BL * 400], f32r, kind="ExternalInput")
    convw_d = nc.dram_tensor("convw", [81, 256], f32r, kind="ExternalInput")
    convb_d = nc.dram_tensor("convb", [128, 2], f32, kind="ExternalInput")
    # prim weights kc-major: [kc 2][pos 81][k 128][mc*128+m 256]
    primw_d = nc.dram_tensor("primw", [2 * 81 * 128 * 256], f32r,
                             kind="ExternalInput")
    primb_d = nc.dram_tensor("primb", [128, 2], f32, kind="ExternalInput")
    w2g_d = nc.dram_tensor("w2g", [9, 128, 16 * CO], f16,
                           kind="ExternalInput")
    biasm_d = nc.dram_tensor("biasm", [128, 1], f32, kind="ExternalInput")
    biast_d = nc.dram_tensor("biast", [128, 1], f32, kind="ExternalInput")
    sela_d = nc.dram_tensor("sela", [128, 160], f32, kind="ExternalInput")
    selt_d = nc.dram_tensor("selt", [128, 160], f32, kind="ExternalInput")
    outm_d = nc.dram_tensor("outm", [160, 32], f32, kind="ExternalOutput")

    # ---- internal DRAM ----
    a2a_in = nc.dram_tensor("a2a_in", [N_CORES * BL * T * 1152], u8)
    a2a_out = nc.dram_tensor("a2a_out", [N_CORES * BL * T * 1152], u8)
    SJP = 5120                 # 4096 main + 1024 folded tail, f32
    sj_in = [nc.dram_tensor(f"sj_in{t}", [SJP], f16) for t in range(T - 1)]
    sj_out = [nc.dram_tensor(f"sj_out{t}", [N_CORES * SJP], f16,
                             addr_space="Shared") for t in range(T - 1)]

    def A(t, p0, pc, dims, foff=0):
        """Raw AP on tile/tensor t: partitions [p0, p0+pc), free dims list
        [(step, count), ...] in elements, plus extra free offset."""
        b = t if isinstance(t, bass.AP) else t[:]
        pitch = b.ap[0][0]
        return bass.AP(b.tensor, b.offset + p0 * pitch + foff,
                       [[pitch, pc]] + [list(d) for d in dims])

    def D(h, dims, off=0):
        """Raw AP on a DRAM tensor handle (flat element space)."""
        b = h[:]
        return bass.AP(b.tensor, off, [list(d) for d in dims])

    def AP2(tl, dims, foff=0):
        """Raw AP on a tile with the partition stride placed anywhere:
        dims entries are (step, count); step 'p' means partition pitch."""
        b = tl[:]
        pitch = b.ap[0][0]
        return bass.AP(b.tensor, b.offset + foff,
                       [[(pitch if s == 'p' else s), c] for s, c in dims])

    rg = [list(range(N_CORES))]

    with tile.TileContext(nc) as tc:
        for _rep in range(reps):
            _run_once(nc, tc, bass, mybir, locals(), stage, solo)

    nc.compile()
    return nc


def _run_once(nc, tc, bass, mybir, env, stage="full", solo=False):
    import numpy as np
    from contextlib import ExitStack
    Alu = mybir.AluOpType
    Act = mybir.ActivationFunctionType
    f32 = mybir.dt.float32
    f32r = mybir.dt.float32r
    f16 = mybir.dt.float16
    u8 = mybir.dt.uint8
    im2_d = env["im2_d"]; convw_d = env["convw_d"]; convb_d = env["convb_d"]
    primw_d = env["primw_d"]; primb_d = env["primb_d"]
    biasm_d = env["biasm_d"]; biast_d = env["biast_d"]
    sela_d = env["sela_d"]; selt_d = env["selt_d"]; outm_d = env["outm_d"]
    a2a_in = env["a2a_in"]; a2a_out = env["a2a_out"]
    w2g_d = env["w2g_d"]
    sj_in = env["sj_in"]; sj_out = env["sj_out"]; SJP = env["SJP"]
    A = env["A"]; D = env["D"]; AP2 = env["AP2"]; rg = env["rg"]

    with ExitStack() as stk:
        persist = stk.enter_context(tc.tile_pool(name="persist", bufs=1))

        # u_hat weights prefetched during phase A (SP-queue DMAs)
        W2C = [persist.tile([128, 16 * CO], f16, name=f"w2c{g}",
                            tag=f"w2c{g}") for g in range(9)]

        # =========== Phase A: conv stage (batch-sharded) ===========
        with ExitStack() as cstk:
            cpool = cstk.enter_context(tc.tile_pool(name="conv", bufs=1))
            wpool = cstk.enter_context(tc.tile_pool(name="wpos", bufs=6))
            cps = cstk.enter_context(tc.tile_pool(name="cpsum", bufs=4,
                                                  space="PSUM"))
            pps = cstk.enter_context(tc.tile_pool(name="ppsum", bufs=1,
                                                  space="PSUM"))

            IM = cpool.tile([81, BL * 400], f32r, name="im", tag="im")
            CW = cpool.tile([81, 256], f32r, name="cw", tag="cw")
            CB = cpool.tile([128, 2], f32, name="cb", tag="cb")
            PB = cpool.tile([128, 2], f32, name="pb", tag="pb")
            nc.sync.dma_start(out=IM[:], in_=im2_d[:])
            nc.sync.dma_start(out=CW[:], in_=convw_d[:])
            nc.sync.dma_start(out=CB[:], in_=convb_d[:])
            nc.sync.dma_start(out=PB[:], in_=primb_d[:])
            for g in range(9):
                nc.sync.dma_start(
                    out=W2C[g][:],
                    in_=D(w2g_d, [(2560, 128), (1, 2560)], g * 128 * 2560))

            # SPIKES[kc] holds conv spikes for all (t, b): [128, 8000] f32r
            SPIKES = [cpool.tile([128, T * BL * 400], f32r, name=f"spk{kc}",
                                 tag=f"spk{kc}") for kc in range(2)]
            CONVOUT = [cpool.tile([128, BL * 400], f32, name=f"co{kc}",
                                  tag=f"co{kc}") for kc in range(2)]
            MPN = [cpool.tile([128, BL * 400], f32, name=f"mpn{kc}",
                              tag=f"mpn{kc}") for kc in range(2)]
            ASC = [cpool.tile([128, BL * 400], f32, name=f"asc{kc}",
                              tag=f"asc{kc}") for kc in range(2)]

            # --- conv1 (f32r): out[co, (b,pix)] ---
            for mc in range(2):
                for b in range(BL):
                    pc = cps.tile([128, 400], f32, name="cvp", tag="cvp")
                    nc.tensor.matmul(
                        out=pc[:, 0:400],
                        lhsT=CW[:, mc * 128:(mc + 1) * 128],
                        rhs=IM[:, b * 400:(b + 1) * 400],
                        start=True, stop=True)
                    nc.vector.tensor_scalar(
                        out=CONVOUT[mc][:, b * 400:(b + 1) * 400],
                        in0=pc[:, 0:400],
                        scalar1=CB[:, mc:mc + 1], scalar2=0.0,
                        op0=Alu.add, op1=Alu.max)

            # --- conv membrane dynamics; the full kc0 chain is emitted
            # first so SPIKES[0] completes early and the PE can start the
            # kc0 prim pass (ASC/is_gt on DVE, MPN on Pool) ---
            for kc in range(2):
                for t in range(T):
                    spk = A(SPIKES[kc], 0, 128, [(1, BL * 400)],
                            t * BL * 400)
                    src = CONVOUT[kc] if t == 0 else ASC[kc]
                    if t > 0:
                        nc.vector.scalar_tensor_tensor(
                            out=ASC[kc][:], in0=MPN[kc][:], scalar=-0.2,
                            in1=CONVOUT[kc][:],
                            op0=Alu.mult, op1=Alu.add)
                    nc.vector.tensor_scalar(
                        out=spk, in0=src[:],
                        scalar1=1.0, scalar2=None, op0=Alu.is_gt)
                    if t < T - 1:
                        nc.gpsimd.tensor_tensor(
                            out=MPN[kc][:], in0=spk.bitcast(f32),
                            in1=src[:], op=Alu.subtract)

            # --- prim conv (f32r), kc-major two passes over 81 positions:
            # out[co, (n=(t,b), oy, ox)] accumulated into 4 psum banks
            PSP = [[pps.tile([128, 360], f32, name=f"pp{mc}{nch}",
                             tag=f"pp{mc}{nch}")
                    for nch in range(2)] for mc in range(2)]
            dmae2 = [nc.sync, nc.scalar]
            for kc in range(2):
                for pos in range(81):
                    ky, kx = pos // 9, pos % 9
                    wt = wpool.tile([128, 256], f32r, name="w", tag="w")
                    dmae2[pos % 2].dma_start(
                        out=wt[:],
                        in_=D(primw_d, [(256, 128), (1, 256)],
                              (kc * 81 + pos) * 128 * 256))
                    for mc in range(2):
                        lhsT = wt[:, mc * 128:(mc + 1) * 128]
                        for nch in range(2):
                            rhs = A(SPIKES[kc], 0, 128,
                                    [(400, 10), (40, 6), (2, 6)],
                                    nch * 4000 + ky * 20 + kx)
                            nc.tensor.matmul(
                                out=PSP[mc][nch][:, 0:360],
                                lhsT=lhsT, rhs=rhs,
                                start=(pos == 0 and kc == 0),
                                stop=(pos == 80 and kc == 1))

            # --- prim evac (+bias), membranes (mc-split), uint8 spikes ---
            PRIM = [cpool.tile([128, T * BL * 36], f32, name=f"pr{mc}",
                               tag=f"pr{mc}") for mc in range(2)]
            PSPK = [cpool.tile([128, T * BL * 36], u8, name=f"ps{mc}",
                               tag=f"ps{mc}") for mc in range(2)]
            MPPN = [cpool.tile([128, BL * 36], f32, name=f"mppn{mc}",
                               tag=f"mppn{mc}") for mc in range(2)]
            APP = [cpool.tile([128, BL * 36], f32, name=f"app{mc}",
                              tag=f"app{mc}") for mc in range(2)]
            for mc in range(2):
                for nch in range(2):
                    nc.vector.tensor_scalar(
                        out=PRIM[mc][:, nch * 360:(nch + 1) * 360],
                        in0=PSP[mc][nch][:, 0:360],
                        scalar1=PB[:, mc:mc + 1], scalar2=None,
                        op0=Alu.add)
            # membranes mc-split across DVE/Pool; pack DMAs interleaved
            # per timestep so the AllToAll payload streams out early.
            # flat f = (mc*128+p)*36 + pix ; dest block j = f//1152.
            # p = ph*32 + pl -> dst = j*23040 + b*5760 + t*1152 + pl*36+pix
            dmal = [nc.sync, nc.scalar]
            di = 0
            for t in range(T):
                for mc, eng in ((0, nc.vector), (1, nc.vector)):
                    po = A(PRIM[mc], 0, 128, [(1, 144)], t * 144)
                    sp = A(PSPK[mc], 0, 128, [(1, 144)], t * 144)
                    src = po if t == 0 else APP[mc][:]
                    if t > 0:
                        eng.scalar_tensor_tensor(
                            out=APP[mc][:], in0=MPPN[mc][:], scalar=-0.2,
                            in1=po, op0=Alu.mult, op1=Alu.add)
                    eng.tensor_scalar(
                        out=sp, in0=src, scalar1=1.0,
                        scalar2=None, op0=Alu.is_gt)
                    if t < T - 1:
                        eng.scalar_tensor_tensor(
                            out=MPPN[mc][:], in0=src, scalar=1.0,
                            in1=src, op0=Alu.is_gt, op1=Alu.subtract)
                for mc in range(2):
                    for ph in range(4):
                        j = mc * 4 + ph
                        src = A(PSPK[mc], ph * 32, 32,
                                [(36, BL), (1, 36)], t * 144)
                        dst = D(a2a_in,
                                [(36, 32), (5760, BL), (1, 36)],
                                j * 23040 + t * 1152)
                        dmal[di % 2].dma_start(out=dst, in_=src)
                        di += 1
            env_spk0 = SPIKES[0]

        if stage == "conv":
            nc.sync.dma_start(out=D(outm_d, [(32, 128), (1, 32)]),
                              in_=A(env_spk0, 0, 128, [(1, 32)]).bitcast(f32))
            return
        # =========== AllToAll: batch-shard -> route-shard ===========
        if solo:
            nc.sync.dma_start(out=a2a_out[:], in_=a2a_in[:])
        else:
            nc.gpsimd.collective_compute(
                "AllToAll", Alu.bypass, replica_groups=rg,
                ins=[a2a_in[:]], outs=[a2a_out[:]])

        # =========== Phase B prep: u_hat (fp16), bias fused in evac ======
        uall = stk.enter_context(tc.tile_pool(name="uall", bufs=1))
        UH_m = uall.tile([128, RL * T * 32], f16, name="uhm", tag="uhm")
        UH_t = uall.tile([128, 36 * T * 32], f16, name="uht", tag="uht")
        BIASM = uall.tile([128, 1], f32, name="biasm", tag="biasm")
        BIAST = uall.tile([128, 1], f32, name="biast", tag="biast")
        nc.sync.dma_start(out=BIASM[:], in_=biasm_d[:])
        nc.sync.dma_start(out=BIAST[:], in_=biast_d[:])
        with ExitStack() as ustk:
            upool = ustk.enter_context(tc.tile_pool(name="uh", bufs=1))
            ups = ustk.enter_context(tc.tile_pool(name="upsum", bufs=4,
                                                  space="PSUM"))

            IDT = upool.tile([32, 32], f16, name="idt", tag="idt")
            from concourse.masks import make_identity
            make_identity(nc, IDT[:])
            for g in range(9):
                # load a2a block, cast to fp16, PE-transpose to X16
                Mg = upool.tile([32, T * 128], u8, name="mg", tag="mg",
                                bufs=2)
                nc.sync.dma_start(
                    out=Mg[:],
                    in_=D(a2a_out, [(5760, 32), (1152, T), (1, 128)],
                          g * 128))
                MgH = upool.tile([32, T * 128], f16, name="mgh", tag="mgh",
                                 bufs=2)
                nc.vector.tensor_copy(out=MgH[:], in_=Mg[:])
                X16 = upool.tile([128, 160], f16, name="x16", tag="x16",
                                 bufs=2)
                for t in range(T):
                    pst = ups.tile([128, 32], f16, name="pst", tag="pst",
                                   bufs=2)
                    nc.tensor.transpose(
                        out=pst[:], in_=MgH[:, t * 128:(t + 1) * 128],
                        identity=IDT[:])
                    (nc.vector.tensor_copy if t % 2 == 0
                     else nc.scalar.copy)(
                        out=X16[:, t * 32:(t + 1) * 32], in_=pst[:])

                # u_hat: K=32 matmuls, r2 order cycles the PE row-group;
                # bias added during evac (DVE tensor_scalar / Act Copy)
                w2c = W2C[g]
                for r2 in (0, 2, 4, 6, 1, 3, 5, 7):
                    psA = ups.tile([128, 320], f32, name="upa", tag="upa",
                                   bufs=3)
                    psB = ups.tile([128, 320], f32, name="upb", tag="upb",
                                   bufs=3)
                    for j in range(2):
                        rr = r2 * 2 + j
                        r = g * 16 + rr
                        rq = r // 36
                        q = (rr // 4) * 32
                        rhs = A(X16, q, 32, [(1, 160)])
                        nc.tensor.matmul(
                            out=psA[:, j * 160:(j + 1) * 160],
                            lhsT=A(w2c, q, 32, [(1, 128)], rr * CO),
                            rhs=rhs, start=True, stop=True,
                            tile_position=(q, 0))
                        nc.tensor.matmul(
                            out=A(psB, rq * 32, 32, [(1, 160)], j * 160),
                            lhsT=A(w2c, q, 32, [(1, 32)], rr * CO + 128),
                            rhs=rhs, start=True, stop=True,
                            tile_position=(q, rq * 32))
                    r0 = g * 16 + r2 * 2
                    rq0, rl0 = r0 // 36, r0 % 36
                    outA = A(UH_m, 0, 128, [(1, 320)], r0 * 160)
                    outB = A(UH_t, rq0 * 32, 32, [(1, 320)], rl0 * 160)
                    inB = A(psB, rq0 * 32, 32, [(1, 320)])
                    biasB = A(BIAST, rq0 * 32, 32, [(1, 1)])
                    ei = (g * 8 + (0, 2, 4, 6, 1, 3, 5, 7).index(r2))
                    eA, eB = [(0, 1), (1, 0)][ei % 2]

                    def _evac(e, o, i, bias):
                        if e == 0:
                            nc.vector.tensor_scalar(
                                out=o, in0=i, scalar1=bias,
                                scalar2=None, op0=Alu.add)
                        elif e == 1:
                            nc.scalar.activation(
                                out=o, in_=i, func=Act.Identity, bias=bias)
                        else:
                            nc.gpsimd.tensor_scalar(
                                out=o, in0=i, scalar1=bias,
                                scalar2=None, op0=Alu.add)
                    _evac(eA, outA, psA[:, 0:320], BIASM[:, 0:1])
                    _evac(eB, outB, inB, biasB)

        if stage == "uhat":
            nc.sync.dma_start(out=D(outm_d, [(32, 128), (1, 32)]),
                              in_=A(UH_m, 0, 128, [(1, 32)]).bitcast(f32))
            return
        # =========== Phase B: digit-caps loop (route-sharded) ========
        dpool = stk.enter_context(tc.tile_pool(name="dig", bufs=1))
        dups = stk.enter_context(tc.tile_pool(name="dups", bufs=2))
        dps = stk.enter_context(tc.tile_pool(name="dpsum", bufs=2,
                                             space="PSUM"))

        NM = RL * 32            # 4608
        NTT = 36 * 32           # 1152
        SELA = dpool.tile([128, 160], f32, name="sela", tag="sela")
        SELT4 = dpool.tile([128, 160], f32, name="selt4", tag="selt4")
        nc.sync.dma_start(out=SELA[:], in_=sela_d[:])
        nc.sync.dma_start(out=SELT4[:], in_=selt_d[:])

        # state tiles
        DS_m = [dpool.tile([128, NM], f16, name=f"dsm{i}", tag=f"dsm{i}")
                for i in range(2)]
        DS_t = [dpool.tile([128, NTT], f16, name=f"dst{i}", tag=f"dst{i}")
                for i in range(2)]
        MDN_m = dpool.tile([128, NM], f16, name="mdnm", tag="mdnm")
        MDN_t = dpool.tile([128, NTT], f16, name="mdnt", tag="mdnt")
        TR_m = dpool.tile([128, NM], f16, name="trm", tag="trm")
        TR_t = dpool.tile([128, NTT], f16, name="trt", tag="trt")
        Y_m = dpool.tile([128, NM], f16, name="ym", tag="ym")
        Y_t = dpool.tile([128, NTT], f16, name="yt", tag="yt")
        BIJ_m = dpool.tile([128, RL], f16, name="bijm", tag="bijm")
        BIJ_t = dpool.tile([128, 36], f16, name="bijt", tag="bijt")
        SJP_m = dpool.tile([128, 32], f32, name="sjpm", tag="sjpm")
        SJPq = dpool.tile([128, 32], f32, name="sjpq", tag="sjpq")
        SJQ4 = dpool.tile([32, 128], f32, name="sjq4", tag="sjq4")
        SJP_t = dpool.tile([32, 32], f32, name="sjpt", tag="sjpt")
        SJ16m = dpool.tile([128, 32], f16, name="sj16m", tag="sj16m")
        SJ16t = dpool.tile([32, 32], f16, name="sj16t", tag="sj16t")
        SJF_m = dpool.tile([128, 32], f32, name="sjfm", tag="sjfm")
        SJF_t = dpool.tile([32, 32], f32, name="sjft", tag="sjft")
        A2_m = dpool.tile([128, 32], f32, name="a2m", tag="a2m")
        A2_t = dpool.tile([32, 32], f32, name="a2t", tag="a2t")
        M2_m = dpool.tile([128, 32], f32, name="m2m", tag="m2m")
        M2_t = dpool.tile([32, 32], f32, name="m2t", tag="m2t")
        D2_m = dpool.tile([128, 32], f16, name="d2m", tag="d2m")
        D2_t = dpool.tile([32, 32], f16, name="d2t", tag="d2t")
        D2R = dpool.tile([128, 32], f16, name="d2r", tag="d2r")
        OUT_m = dpool.tile([128, 32], f32, name="outm", tag="outm")
        OUT_t = dpool.tile([32, 32], f32, name="outt", tag="outt")
        ZB_m = dpool.tile([128, RL], f32, name="zbm", tag="zbm")
        ZB_t = dpool.tile([128, 36], f32, name="zbt", tag="zbt")
        DPDF = dpool.tile([128, 36], f32, name="dpdf", tag="dpdf")

        bij0 = float(np.float32(1.0) / np.float32(R))
        nc.vector.memset(BIJ_m[:], bij0)
        nc.vector.memset(BIJ_t[:], bij0)
        NEG1 = dpool.tile([128, 1], f32, name="neg1", tag="neg1")
        ZERO = dpool.tile([128, 1], f32, name="zero", tag="zero")
        nc.vector.memset(NEG1[:], -1.0)
        nc.vector.memset(ZERO[:], 0.0)

        HR = 72                 # r-split point for z/tree engine split

        def Um(t):
            return A(UH_m, 0, 128, [(5 * 32, RL), (1, 32)], t * 32)

        def Ut(t):
            return A(UH_t, 0, 128, [(5 * 32, 36), (1, 32)], t * 32)

        def spikes(t, dsm, dst):
            """DS = relu(sign(U - 1)) on the Act engine; v5/v6 HW runs
            had bit-identical output either way, so the tables are safe
            here and this keeps the during-AG work off DVE."""
            nc.scalar.activation(out=dsm[:], in_=Um(t), func=Act.Sign,
                                 bias=NEG1[:, 0:1])
            nc.scalar.activation(out=dsm[:], in_=dsm[:], func=Act.Relu,
                                 bias=ZERO[:, 0:1])
            nc.scalar.activation(out=dst[:], in_=Ut(t), func=Act.Sign,
                                 bias=NEG1[:, 0:1])
            nc.scalar.activation(out=dst[:], in_=dst[:], func=Act.Relu,
                                 bias=ZERO[:, 0:1])

        # ---- preamble: t=0 spikes / reset / trace ----
        spikes(0, DS_m[0], DS_t[0])
        nc.vector.tensor_tensor(out=MDN_m[:], in0=Um(0), in1=DS_m[0][:],
                                op=Alu.subtract)
        nc.vector.tensor_tensor(out=MDN_t[:], in0=Ut(0), in1=DS_t[0][:],
                                op=Alu.subtract)
        nc.scalar.activation(out=TR_m[:], in_=DS_m[0][:], func=Act.Copy)
        nc.scalar.activation(out=TR_t[:], in_=DS_t[0][:], func=Act.Copy)

        for t in range(T):
            cm, ct = DS_m[t % 2], DS_t[t % 2]
            nm, nt = DS_m[(t + 1) % 2], DS_t[(t + 1) % 2]
            Ymv = A(Y_m, 0, 128, [(32, RL), (1, 32)])
            Ytv = A(Y_t, 0, 128, [(32, 36), (1, 32)])

            # ---- y = ds * bij (Pool), s_j partials (DVE) ----
            nc.gpsimd.tensor_tensor(
                out=Ymv, in0=A(cm, 0, 128, [(32, RL), (1, 32)]),
                in1=A(BIJ_m, 0, 128, [(1, RL), (0, 32)]), op=Alu.mult)
            nc.gpsimd.tensor_tensor(
                out=Ytv, in0=A(ct, 0, 128, [(32, 36), (1, 32)]),
                in1=A(BIJ_t, 0, 128, [(1, 36), (0, 32)]), op=Alu.mult)
            nc.vector.tensor_reduce(
                out=SJP_m[:], in_=A(Y_m, 0, 128, [(1, 32), (32, RL)]),
                axis=mybir.AxisListType.X, op=Alu.add)
            nc.vector.tensor_reduce(
                out=SJPq[:], in_=A(Y_t, 0, 128, [(1, 32), (32, 36)]),
                axis=mybir.AxisListType.X, op=Alu.add)
            # fold tail partial [128 (rq,cot), 32] -> [32 cot, (rq 4, 32)]
            for rq in range(4):
                (nc.sync if rq % 2 else nc.gpsimd).dma_start(
                    out=A(SJQ4, 0, 32, [(1, 32)], rq * 32),
                    in_=A(SJPq, rq * 32, 32, [(1, 32)]))
            nc.vector.tensor_reduce(
                out=SJP_t[:], in_=A(SJQ4, 0, 32, [(1, 32), (32, 4)]),
                axis=mybir.AxisListType.X, op=Alu.add)
            # local out_mem accumulation (host sums cores)
            if t == 0:
                nc.vector.tensor_copy(out=OUT_m[:], in_=SJP_m[:])
                nc.vector.tensor_copy(out=OUT_t[:], in_=SJP_t[:])
            else:
                nc.vector.tensor_tensor(out=OUT_m[:], in0=OUT_m[:],
                                        in1=SJP_m[:], op=Alu.add)
                nc.vector.tensor_tensor(out=OUT_t[:], in0=OUT_t[:],
                                        in1=SJP_t[:], op=Alu.add)
            if t == T - 1:
                break

            nc.vector.tensor_copy(out=SJ16m[:], in_=SJP_m[:])
            nc.vector.tensor_copy(out=SJ16t[:], in_=SJP_t[:])
            nc.sync.dma_start(out=D(sj_in[t], [(32, 128), (1, 32)]),
                              in_=SJ16m[:])
            nc.sync.dma_start(out=D(sj_in[t], [(32, 32), (1, 32)], 4096),
                              in_=SJ16t[:])

            # ---- a-pass for t+1 on DVE (runs during the collective;
            # Pool is blocked while the gpsimd queue owns the AG) ----
            nc.vector.scalar_tensor_tensor(
                out=Um(t + 1), in0=MDN_m[:], scalar=0.2,
                in1=Um(t + 1), op0=Alu.mult, op1=Alu.add)
            nc.vector.scalar_tensor_tensor(
                out=Ut(t + 1), in0=MDN_t[:], scalar=0.2,
                in1=Ut(t + 1), op0=Alu.mult, op1=Alu.add)
            # trace update for this step, in the during-AG DVE slot; z of
            # this step reads it after the AllGather
            if 0 < t < T - 1:
                nc.vector.scalar_tensor_tensor(
                    out=TR_m[:], in0=TR_m[:], scalar=float(DECAY_TR),
                    in1=cm[:], op0=Alu.mult, op1=Alu.max)
                nc.vector.scalar_tensor_tensor(
                    out=TR_t[:], in0=TR_t[:], scalar=float(DECAY_TR),
                    in1=ct[:], op0=Alu.mult, op1=Alu.max)

            if solo:
                nc.sync.dma_start(
                    out=D(sj_out[t], [(1, SJP)]), in_=sj_in[t][:])
            else:
                nc.gpsimd.collective_compute(
                    "AllGather", Alu.bypass, replica_groups=rg,
                    ins=[sj_in[t][:]], outs=[sj_out[t][:]])

            # ---- during-AG: spikes t+1 on Act ----
            spikes(t + 1, nm, nt)

            # ---- post-AG: gather s_j, dig2 membranes ----
            SJG_m = dups.tile([128, 8 * 32], f16, name="sjgm", tag="sjgm")
            SJG_t = dups.tile([32, 8 * 32], f16, name="sjgt", tag="sjgt")
            nc.sync.dma_start(
                out=A(SJG_m, 0, 128, [(32, 8), (1, 32)]),
                in_=D(sj_out[t], [(32, 128), (SJP, 8), (1, 32)]))
            nc.scalar.dma_start(
                out=A(SJG_t, 0, 32, [(32, 8), (1, 32)]),
                in_=D(sj_out[t], [(32, 32), (SJP, 8), (1, 32)], 4096))
            nc.vector.tensor_reduce(
                out=SJF_m[:], in_=A(SJG_m, 0, 128, [(1, 32), (32, 8)]),
                axis=mybir.AxisListType.X, op=Alu.add)
            nc.vector.tensor_reduce(
                out=SJF_t[:], in_=A(SJG_t, 0, 32, [(1, 32), (32, 8)]),
                axis=mybir.AxisListType.X, op=Alu.add)

            if t == 0:
                a2m, a2t = SJF_m, SJF_t
            else:
                nc.vector.scalar_tensor_tensor(
                    out=A2_m[:], in0=M2_m[:], scalar=0.2, in1=SJF_m[:],
                    op0=Alu.mult, op1=Alu.add)
                nc.vector.scalar_tensor_tensor(
                    out=A2_t[:], in0=M2_t[:], scalar=0.2, in1=SJF_t[:],
                    op0=Alu.mult, op1=Alu.add)
                a2m, a2t = A2_m, A2_t
            nc.vector.tensor_scalar(
                out=D2_m[:], in0=a2m[:], scalar1=0.5, scalar2=None,
                op0=Alu.is_gt)
            nc.vector.tensor_scalar(
                out=D2_t[:], in0=a2t[:], scalar1=0.5, scalar2=None,
                op0=Alu.is_gt)
            for rq in range(4):
                (nc.sync if rq % 2 else nc.scalar).dma_start(
                    out=A(D2R, rq * 32, 32, [(1, 32)]), in_=D2_t[:])
            if t < T - 2:
                nc.vector.scalar_tensor_tensor(
                    out=M2_m[:], in0=D2_m[:], scalar=-0.5, in1=a2m[:],
                    op0=Alu.mult, op1=Alu.add)
                nc.vector.scalar_tensor_tensor(
                    out=M2_t[:], in0=D2_t[:], scalar=-0.5, in1=a2t[:],
                    op0=Alu.mult, op1=Alu.add)

            # ---- z = (trace - 0.1) * d2s (r-split DVE/Pool onto Y) ----
            nc.vector.scalar_tensor_tensor(
                out=A(Y_m, 0, 128, [(32, HR), (1, 32)]),
                in0=A(TR_m, 0, 128, [(32, HR), (1, 32)]),
                scalar=-0.1, in1=A(D2_m, 0, 128, [(0, HR), (1, 32)]),
                op0=Alu.add, op1=Alu.mult)
            nc.vector.scalar_tensor_tensor(
                out=A(Y_m, 0, 128, [(32, RL - HR), (1, 32)], HR * 32),
                in0=A(TR_m, 0, 128, [(32, RL - HR), (1, 32)], HR * 32),
                scalar=-0.1,
                in1=A(D2_m, 0, 128, [(0, RL - HR), (1, 32)]),
                op0=Alu.add, op1=Alu.mult)
            nc.vector.scalar_tensor_tensor(
                out=Ytv, in0=A(TR_t, 0, 128, [(32, 36), (1, 32)]),
                scalar=-0.1, in1=A(D2R, 0, 128, [(0, 36), (1, 32)]),
                op0=Alu.add, op1=Alu.mult)
            # fold b 32->8 (tree), then reduce -> ZB[co, r]
            for w in (16, 8):
                nc.vector.tensor_tensor(
                    out=A(Y_m, 0, 128, [(32, HR), (1, w)]),
                    in0=A(Y_m, 0, 128, [(32, HR), (1, w)]),
                    in1=A(Y_m, 0, 128, [(32, HR), (1, w)], w),
                    op=Alu.add)
                nc.gpsimd.tensor_tensor(
                    out=A(Y_m, 0, 128, [(32, RL - HR), (1, w)], HR * 32),
                    in0=A(Y_m, 0, 128, [(32, RL - HR), (1, w)], HR * 32),
                    in1=A(Y_m, 0, 128, [(32, RL - HR), (1, w)],
                          HR * 32 + w),
                    op=Alu.add)
                nc.gpsimd.tensor_tensor(
                    out=A(Y_t, 0, 128, [(32, 36), (1, w)]),
                    in0=A(Y_t, 0, 128, [(32, 36), (1, w)]),
                    in1=A(Y_t, 0, 128, [(32, 36), (1, w)], w),
                    op=Alu.add)
            nc.vector.tensor_reduce(
                out=A(ZB_m, 0, 128, [(1, HR)]),
                in_=A(Y_m, 0, 128, [(32, HR), (1, 8)]),
                axis=mybir.AxisListType.X, op=Alu.add)
            nc.vector.tensor_reduce(
                out=A(ZB_m, 0, 128, [(1, RL - HR)], HR),
                in_=A(Y_m, 0, 128, [(32, RL - HR), (1, 8)], HR * 32),
                axis=mybir.AxisListType.X, op=Alu.add)
            nc.vector.tensor_reduce(
                out=ZB_t[:], in_=A(Y_t, 0, 128, [(32, 36), (1, 8)]),
                axis=mybir.AxisListType.X, op=Alu.add)
            # ---- delta matmuls: PD[co', r] = sum_co sel * zb; the co
            # tail accumulates per rq-block straight from ZB_t ----
            PD_m = dps.tile([128, 145], f32, name="pdm", tag="pdm")
            PD_t = dps.tile([32, 145], f32, name="pdt", tag="pdt")
            for rq in range(4):
                cs = slice(rq * 36, (rq + 1) * 36)
                nc.tensor.matmul(out=PD_m[:, cs],
                                 lhsT=SELA[:, 0:128], rhs=ZB_m[:, cs],
                                 start=True, stop=False)
                nc.tensor.matmul(
                    out=PD_m[:, cs],
                    lhsT=A(SELT4, rq * 32, 32, [(1, 128)]),
                    rhs=A(ZB_t, rq * 32, 32, [(1, 36)]),
                    start=False, stop=True,
                    tile_position=(rq * 32, 0))
                nc.tensor.matmul(out=PD_t[:, cs],
                                 lhsT=SELA[:, 128:160], rhs=ZB_m[:, cs],
                                 start=True, stop=False)
                nc.tensor.matmul(
                    out=PD_t[:, cs],
                    lhsT=A(SELT4, rq * 32, 32, [(1, 32)], 128),
                    rhs=A(ZB_t, rq * 32, 32, [(1, 36)]),
                    start=False, stop=True,
                    tile_position=(rq * 32, 0))

            # ---- bij updates; tail folded via one PSUM->SBUF DMA ----
            nc.vector.scalar_tensor_tensor(
                out=BIJ_m[:], in0=PD_m[:, 0:144], scalar=float(ALPHA),
                in1=BIJ_m[:], op0=Alu.mult, op1=Alu.add)
            PDTS = dups.tile([32, 145], f32, name="pdts", tag="pdts")
            nc.scalar.copy(out=PDTS[:, 0:144], in_=PD_t[:, 0:144])
            for rq in range(4):
                (nc.sync if rq % 2 else nc.scalar).dma_start(
                    out=A(DPDF, rq * 32, 32, [(1, 36)]),
                    in_=A(PDTS, 0, 32, [(1, 36)], rq * 36))
            nc.vector.scalar_tensor_tensor(
                out=BIJ_t[:], in0=DPDF[:], scalar=float(ALPHA),
                in1=BIJ_t[:], op0=Alu.mult, op1=Alu.add)

            # ---- slack: reset (Pool, TensorTensor) for t+1 ----
            if t < T - 2:
                nc.gpsimd.tensor_tensor(out=MDN_m[:], in0=Um(t + 1),
                                        in1=nm[:], op=Alu.subtract)
                nc.gpsimd.tensor_tensor(out=MDN_t[:], in0=Ut(t + 1),
                                        in1=nt[:], op=Alu.subtract)

        # ---- write local-partial outputs ----
        nc.sync.dma_start(out=D(outm_d, [(32, 128), (1, 32)]),
                          in_=OUT_m[:])
        nc.sync.dma_start(out=D(outm_d, [(32, 32), (1, 32)], 128 * 32),
                          in_=OUT_t[:])


def _host_prepare(data, conv_w, conv_b, prim_w, prim_b, W, bias):
    """Build per-core input maps."""
    from numpy.lib.stride_tricks import sliding_window_view
    f32 = np.float32
    data = np.asarray(data, f32)
    conv_w = np.asarray(conv_w, f32)
    conv_b = np.asarray(conv_b, f32)
    prim_w = np.asarray(prim_w, f32)
    prim_b = np.asarray(prim_b, f32)
    W = np.asarray(W, f32)
    bias = np.asarray(bias, f32)

    # im2col: win[b, ky, kx, oy, ox]
    win = sliding_window_view(data[:, 0, :, :], (20, 20), axis=(1, 2))
    im2_all = np.ascontiguousarray(win).reshape(B, 81, 400)

    # everything feeding the spiking membranes runs in a 2x-scaled domain
    # (exact in fp32) so the reset is the plain subtract M = A - ds.
    convw = np.ascontiguousarray(conv_w.reshape(256, 81).T) * f32(2.0)
    convb2 = np.ascontiguousarray(conv_b.reshape(2, 128).T) * f32(2.0)

    # prim weights kc-major: [kc, pos, k, mc*128+m]
    pw = prim_w.reshape(2, 128, 2, 128, 9, 9)   # [mc, m, kc, k, ky, kx]
    primw = np.ascontiguousarray(
        pw.transpose(2, 4, 5, 3, 0, 1).reshape(2 * 81 * 128 * 256)) \
        * f32(2.0)
    primb2 = np.ascontiguousarray(prim_b.reshape(2, 128).T) * f32(2.0)

    # W2[i, r, co] with co = o*10 + c, zero-padded to K=32 route-quads:
    # w2g[g, rr*8+i, rr*160+co] = 2*W2[i, g*16+rr, co]  (fp16)
    Wt = np.ascontiguousarray(
        W.transpose(3, 0, 2, 1)).reshape(8, R, CO) * f32(2.0)

    bias_o = bias[:, 0]
    bias2 = np.array([f32(2.0) * bias_o[co // 10] for co in range(CO)], f32)
    biasm = np.ascontiguousarray(bias2[:128].reshape(128, 1))
    biast = np.zeros((128, 1), f32)
    for p in range(128):
        biast[p, 0] = bias2[128 + p % 32]

    cos = np.arange(CO)
    sela = (np.equal.outer(cos[:128] % 10, cos % 10)).astype(f32)
    selt = (np.equal.outer(cos[128:] % 10, cos % 10)).astype(f32)
    sela = np.ascontiguousarray(sela)
    selt = np.ascontiguousarray(np.tile(selt, (4, 1)))

    in_maps = []
    for k in range(N_CORES):
        im2 = np.ascontiguousarray(
            im2_all[BL * k:BL * (k + 1)].transpose(1, 0, 2).reshape(81, 1600))
        w2core = Wt[:, RL * k:RL * (k + 1), :]          # [8, 144, 160]
        w2g = np.zeros((9, 128, 16 * CO), np.float16)
        for rr in range(16):
            blk = w2core[:, rr::16, :]
            w2g[:, rr * 8:(rr + 1) * 8, rr * CO:(rr + 1) * CO] = \
                blk.transpose(1, 0, 2).astype(np.float16)
        in_maps.append({
            "im2": im2, "convw": convw, "convb": convb2,
            "primw": primw, "primb": primb2, "w2g": w2g,
            "biasm": biasm, "biast": biast, "sela": sela, "selt": selt,
        })
    return in_maps


def _postprocess(outs):
    """sum per-core outm [160, 32] partials -> classes [32, 10]."""
    outm = np.zeros((160, 32), np.float32)
    for o in outs:
        outm += np.asarray(o, np.float32)
    out3 = outm.reshape(16, 10, 32) / np.float32(T)
    sq = (out3 * out3).sum(axis=0)
    return np.sqrt(sq).T.astype(np.float32)


def kernel(data, conv_w, conv_b, prim_w, prim_b, W, bias, time_window):
    from concourse.bass_utils import run_bass_kernel_spmd
    assert int(time_window) == T
    if "nc" not in _CACHE:
        _CACHE["nc"] = _build_program()
    nc = _CACHE["nc"]
    in_maps = _host_prepare(data, conv_w, conv_b, prim_w, prim_b, W, bias)
    res = run_bass_kernel_spmd(nc, in_maps, core_ids=list(range(N_CORES)))
    return _postprocess([r["outm"] for r in res.results])


# revision 29
# speedup vs baseline: 1.2007x; 1.2007x over previous
"""Trainium2 Bass kernel for the spiking CapsNet forward pass (nn_CapsNet).

v2 strategy (8 NeuronCores):
  Phase A (batch-parallel, 4 images/core):
    conv1 once in f32r, conv-layer membrane dynamics for 5 steps
    (kc0 spike chain prioritized so the PE can start early), then the
    prim conv kc-major in two weight passes over all 5 timesteps.
  AllToAll: prim spikes as uint8, batch-shard -> route-shard (184KB).
  Phase B (route-parallel, 144 routes/core, full batch):
    u_hat on PE in fp16 with the conv bias fused into the PSUM
    evacuation (DVE/Act alternating); digit-caps loop with the
    membrane chain (a-pass / spikes / reset / trace) scheduled to run
    during each step's AllGather: spikes on the Activation engine
    (Sign+Relu), a-pass on Pool before the collective, reset/trace as
    slack work after the routing chain.  s_j partials are exchanged
    with a 5120-float AllGather at t=0..3 only; out_mem accumulates
    LOCAL route partials each step and the host sums the 8 cores.
Host: im2col + weight relayouts, final classes reduction.
"""

import numpy as np

N_CORES = 8
T = 5
B = 32
BL = B // N_CORES          # local batch (4)
R = 1152
RL = R // N_CORES          # local routes (144)
CO = 160                   # (o,c) pairs, ordered co = o*10 + c
DECAY_TR = np.float32(np.exp(np.float32(-1.0 / 1.5)))
ALPHA = np.float32(np.float32(0.0008) / np.float32(32.0))

_CACHE = {}


def _build_program(reps=1, stage="full", solo=False):
    import concourse.bass as bass
    import concourse.mybir as mybir
    import concourse.tile as tile
    from concourse import bacc

    f32 = mybir.dt.float32
    f32r = mybir.dt.float32r
    f16 = mybir.dt.float16
    u8 = mybir.dt.uint8

    nc = bacc.Bacc("TRN2", target_bir_lowering=False, debug=False,
                   num_devices=N_CORES)

    # ---- external I/O (per-core values supplied by host) ----
    im2_d = nc.dram_tensor("im2", [81, BL * 400], f32r, kind="ExternalInput")
    convw_d = nc.dram_tensor("convw", [81, 256], f32r, kind="ExternalInput")
    convb_d = nc.dram_tensor("convb", [128, 2], f32, kind="ExternalInput")
    # prim weights kc-major: [kc 2][pos 81][k 128][mc*128+m 256]
    primw_d = nc.dram_tensor("primw", [2 * 81 * 128 * 256], f32r,
                             kind="ExternalInput")
    primb_d = nc.dram_tensor("primb", [128, 2], f32, kind="ExternalInput")
    w2g_d = nc.dram_tensor("w2g", [9, 128, 16 * CO], f16,
                           kind="ExternalInput")
    biasm_d = nc.dram_tensor("biasm", [128, 1], f32, kind="ExternalInput")
    biast_d = nc.dram_tensor("biast", [128, 1], f32, kind="ExternalInput")
    sela_d = nc.dram_tensor("sela", [128, 160], f32, kind="ExternalInput")
    selt_d = nc.dram_tensor("selt", [128, 160], f32, kind="ExternalInput")
    outm_d = nc.dram_tensor("outm", [160, 32], f32, kind="ExternalOutput")

    # ---- internal DRAM ----
    a2a_in = nc.dram_tensor("a2a_in", [N_CORES * BL * T * 1152], u8)
    a2a_out = nc.dram_tensor("a2a_out", [N_CORES * BL * T * 1152], u8)
    SJP = 5120                 # 4096 main + 1024 folded tail, f32
    sj_in = [nc.dram_tensor(f"sj_in{t}", [SJP], f16) for t in range(T - 1)]
    sj_out = [nc.dram_tensor(f"sj_out{t}", [N_CORES * SJP], f16,
                             addr_space="Shared") for t in range(T - 1)]

    def A(t, p0, pc, dims, foff=0):
        """Raw AP on tile/tensor t: partitions [p0, p0+pc), free dims list
        [(step, count), ...] in elements, plus extra free offset."""
        b = t if isinstance(t, bass.AP) else t[:]
        pitch = b.ap[0][0]
        return bass.AP(b.tensor, b.offset + p0 * pitch + foff,
                       [[pitch, pc]] + [list(d) for d in dims])

    def D(h, dims, off=0):
        """Raw AP on a DRAM tensor handle (flat element space)."""
        b = h[:]
        return bass.AP(b.tensor, off, [list(d) for d in dims])

    def AP2(tl, dims, foff=0):
        """Raw AP on a tile with the partition stride placed anywhere:
        dims entries are (step, count); step 'p' means partition pitch."""
        b = tl[:]
        pitch = b.ap[0][0]
        return bass.AP(b.tensor, b.offset + foff,
                       [[(pitch if s == 'p' else s), c] for s, c in dims])

    rg = [list(range(N_CORES))]

    with tile.TileContext(nc) as tc:
        for _rep in range(reps):
            _run_once(nc, tc, bass, mybir, locals(), stage, solo)

    nc.compile()
    return nc


def _run_once(nc, tc, bass, mybir, env, stage="full", solo=False):
    import numpy as np
    from contextlib import ExitStack
    Alu = mybir.AluOpType
    Act = mybir.ActivationFunctionType
    f32 = mybir.dt.float32
    f32r = mybir.dt.float32r
    f16 = mybir.dt.float16
    u8 = mybir.dt.uint8
    im2_d = env["im2_d"]; convw_d = env["convw_d"]; convb_d = env["convb_d"]
    primw_d = env["primw_d"]; primb_d = env["primb_d"]
    biasm_d = env["biasm_d"]; biast_d = env["biast_d"]
    sela_d = env["sela_d"]; selt_d = env["selt_d"]; outm_d = env["outm_d"]
    a2a_in = env["a2a_in"]; a2a_out = env["a2a_out"]
    w2g_d = env["w2g_d"]
    sj_in = env["sj_in"]; sj_out = env["sj_out"]; SJP = env["SJP"]
    A = env["A"]; D = env["D"]; AP2 = env["AP2"]; rg = env["rg"]

    with ExitStack() as stk:
        persist = stk.enter_context(tc.tile_pool(name="persist", bufs=1))

        # u_hat weights prefetched during phase A (SP-queue DMAs)
        W2C = [persist.tile([128, 16 * CO], f16, name=f"w2c{g}",
                            tag=f"w2c{g}") for g in range(9)]

        # =========== Phase A: conv stage (batch-sharded) ===========
        with ExitStack() as cstk:
            cpool = cstk.enter_context(tc.tile_pool(name="conv", bufs=1))
            wpool = cstk.enter_context(tc.tile_pool(name="wpos", bufs=6))
            cps = cstk.enter_context(tc.tile_pool(name="cpsum", bufs=4,
                                                  space="PSUM"))
            pps = cstk.enter_context(tc.tile_pool(name="ppsum", bufs=1,
                                                  space="PSUM"))

            IM = cpool.tile([81, BL * 400], f32r, name="im", tag="im")
            CW = cpool.tile([81, 256], f32r, name="cw", tag="cw")
            CB = cpool.tile([128, 2], f32, name="cb", tag="cb")
            PB = cpool.tile([128, 2], f32, name="pb", tag="pb")
            nc.sync.dma_start(out=IM[:], in_=im2_d[:])
            nc.sync.dma_start(out=CW[:], in_=convw_d[:])
            nc.sync.dma_start(out=CB[:], in_=convb_d[:])
            nc.sync.dma_start(out=PB[:], in_=primb_d[:])
            for g in range(9):
                nc.sync.dma_start(
                    out=W2C[g][:],
                    in_=D(w2g_d, [(2560, 128), (1, 2560)], g * 128 * 2560))

            # SPIKES[kc] holds conv spikes for all (t, b): [128, 8000] f32r
            SPIKES = [cpool.tile([128, T * BL * 400], f32r, name=f"spk{kc}",
                                 tag=f"spk{kc}") for kc in range(2)]
            CONVOUT = [cpool.tile([128, BL * 400], f32, name=f"co{kc}",
                                  tag=f"co{kc}") for kc in range(2)]
            MPN = [cpool.tile([128, BL * 400], f32, name=f"mpn{kc}",
                              tag=f"mpn{kc}") for kc in range(2)]
            ASC = [cpool.tile([128, BL * 400], f32, name=f"asc{kc}",
                              tag=f"asc{kc}") for kc in range(2)]

            # --- conv1 (f32r): out[co, (b,pix)] ---
            for mc in range(2):
                for b in range(BL):
                    pc = cps.tile([128, 400], f32, name="cvp", tag="cvp")
                    nc.tensor.matmul(
                        out=pc[:, 0:400],
                        lhsT=CW[:, mc * 128:(mc + 1) * 128],
                        rhs=IM[:, b * 400:(b + 1) * 400],
                        start=True, stop=True)
                    nc.vector.tensor_scalar(
                        out=CONVOUT[mc][:, b * 400:(b + 1) * 400],
                        in0=pc[:, 0:400],
                        scalar1=CB[:, mc:mc + 1], scalar2=0.0,
                        op0=Alu.add, op1=Alu.max)

            # --- conv membrane dynamics; the full kc0 chain is emitted
            # first so SPIKES[0] completes early and the PE can start the
            # kc0 prim pass (ASC/is_gt on DVE, MPN on Pool) ---
            for kc in range(2):
                for t in range(T):
                    spk = A(SPIKES[kc], 0, 128, [(1, BL * 400)],
                            t * BL * 400)
                    src = CONVOUT[kc] if t == 0 else ASC[kc]
                    if t > 0:
                        nc.vector.scalar_tensor_tensor(
                            out=ASC[kc][:], in0=MPN[kc][:], scalar=-0.2,
                            in1=CONVOUT[kc][:],
                            op0=Alu.mult, op1=Alu.add)
                    nc.vector.tensor_scalar(
                        out=spk, in0=src[:],
                        scalar1=1.0, scalar2=None, op0=Alu.is_gt)
                    if t < T - 1:
                        nc.gpsimd.tensor_tensor(
                            out=MPN[kc][:], in0=spk.bitcast(f32),
                            in1=src[:], op=Alu.subtract)

            # --- prim conv (f32r), kc-major two passes over 81 positions:
            # out[co, (n=(t,b), oy, ox)] accumulated into 4 psum banks
            PSP = [[pps.tile([128, 360], f32, name=f"pp{mc}{nch}",
                             tag=f"pp{mc}{nch}")
                    for nch in range(2)] for mc in range(2)]
            dmae2 = [nc.sync, nc.scalar]
            for kc in range(2):
                for pos in range(81):
                    ky, kx = pos // 9, pos % 9
                    wt = wpool.tile([128, 256], f32r, name="w", tag="w")
                    dmae2[pos % 2].dma_start(
                        out=wt[:],
                        in_=D(primw_d, [(256, 128), (1, 256)],
                              (kc * 81 + pos) * 128 * 256))
                    for mc in range(2):
                        lhsT = wt[:, mc * 128:(mc + 1) * 128]
                        for nch in range(2):
                            rhs = A(SPIKES[kc], 0, 128,
                                    [(400, 10), (40, 6), (2, 6)],
                                    nch * 4000 + ky * 20 + kx)
                            nc.tensor.matmul(
                                out=PSP[mc][nch][:, 0:360],
                                lhsT=lhsT, rhs=rhs,
                                start=(pos == 0 and kc == 0),
                                stop=(pos == 80 and kc == 1))

            # --- prim evac (+bias), membranes (mc-split), uint8 spikes ---
            PRIM = [cpool.tile([128, T * BL * 36], f32, name=f"pr{mc}",
                               tag=f"pr{mc}") for mc in range(2)]
            PSPK = [cpool.tile([128, T * BL * 36], u8, name=f"ps{mc}",
                               tag=f"ps{mc}") for mc in range(2)]
            MPPN = [cpool.tile([128, BL * 36], f32, name=f"mppn{mc}",
                               tag=f"mppn{mc}") for mc in range(2)]
            APP = [cpool.tile([128, BL * 36], f32, name=f"app{mc}",
                              tag=f"app{mc}") for mc in range(2)]
            for mc in range(2):
                for nch in range(2):
                    nc.vector.tensor_scalar(
                        out=PRIM[mc][:, nch * 360:(nch + 1) * 360],
                        in0=PSP[mc][nch][:, 0:360],
                        scalar1=PB[:, mc:mc + 1], scalar2=None,
                        op0=Alu.add)
            # membranes mc-split across DVE/Pool; pack DMAs interleaved
            # per timestep so the AllToAll payload streams out early.
            # flat f = (mc*128+p)*36 + pix ; dest block j = f//1152.
            # p = ph*32 + pl -> dst = j*23040 + b*5760 + t*1152 + pl*36+pix
            dmal = [nc.sync, nc.scalar]
            di = 0
            for t in range(T):
                for mc, eng in ((0, nc.vector), (1, nc.vector)):
                    po = A(PRIM[mc], 0, 128, [(1, 144)], t * 144)
                    sp = A(PSPK[mc], 0, 128, [(1, 144)], t * 144)
                    src = po if t == 0 else APP[mc][:]
                    if t > 0:
                        eng.scalar_tensor_tensor(
                            out=APP[mc][:], in0=MPPN[mc][:], scalar=-0.2,
                            in1=po, op0=Alu.mult, op1=Alu.add)
                    eng.tensor_scalar(
                        out=sp, in0=src, scalar1=1.0,
                        scalar2=None, op0=Alu.is_gt)
                    if t < T - 1:
                        eng.scalar_tensor_tensor(
                            out=MPPN[mc][:], in0=src, scalar=1.0,
                            in1=src, op0=Alu.is_gt, op1=Alu.subtract)
                for mc in range(2):
                    for ph in range(4):
                        j = mc * 4 + ph
                        src = A(PSPK[mc], ph * 32, 32,
                                [(36, BL), (1, 36)], t * 144)
                        dst = D(a2a_in,
                                [(36, 32), (5760, BL), (1, 36)],
                                j * 23040 + t * 1152)
                        dmal[di % 2].dma_start(out=dst, in_=src)
                        di += 1
            env_spk0 = SPIKES[0]

        if stage == "conv":
            nc.sync.dma_start(out=D(outm_d, [(32, 128), (1, 32)]),
                              in_=A(env_spk0, 0, 128, [(1, 32)]).bitcast(f32))
            return
        # =========== AllToAll: batch-shard -> route-shard ===========
        if solo:
            nc.sync.dma_start(out=a2a_out[:], in_=a2a_in[:])
        else:
            nc.gpsimd.collective_compute(
                "AllToAll", Alu.bypass, replica_groups=rg,
                ins=[a2a_in[:]], outs=[a2a_out[:]])

        # =========== Phase B prep: u_hat (fp16), bias fused in evac ======
        uall = stk.enter_context(tc.tile_pool(name="uall", bufs=1))
        UH_m = uall.tile([128, RL * T * 32], f16, name="uhm", tag="uhm")
        UH_t = uall.tile([128, 36 * T * 32], f16, name="uht", tag="uht")
        BIASM = uall.tile([128, 1], f32, name="biasm", tag="biasm")
        BIAST = uall.tile([128, 1], f32, name="biast", tag="biast")
        nc.sync.dma_start(out=BIASM[:], in_=biasm_d[:])
        nc.sync.dma_start(out=BIAST[:], in_=biast_d[:])
        with ExitStack() as ustk:
            upool = ustk.enter_context(tc.tile_pool(name="uh", bufs=1))
            ups = ustk.enter_context(tc.tile_pool(name="upsum", bufs=4,
                                                  space="PSUM"))

            IDT = upool.tile([32, 32], f16, name="idt", tag="idt")
            from concourse.masks import make_identity
            make_identity(nc, IDT[:])
            for g in range(9):
                # load a2a block, cast to fp16, PE-transpose to X16
                Mg = upool.tile([32, T * 128], u8, name="mg", tag="mg",
                                bufs=2)
                nc.sync.dma_start(
                    out=Mg[:],
                    in_=D(a2a_out, [(5760, 32), (1152, T), (1, 128)],
                          g * 128))
                MgH = upool.tile([32, T * 128], f16, name="mgh", tag="mgh",
                                 bufs=2)
                nc.vector.tensor_copy(out=MgH[:], in_=Mg[:])
                X16 = upool.tile([128, 160], f16, name="x16", tag="x16",
                                 bufs=2)
                for t in range(T):
                    pst = ups.tile([128, 32], f16, name="pst", tag="pst",
                                   bufs=2)
                    nc.tensor.transpose(
                        out=pst[:], in_=MgH[:, t * 128:(t + 1) * 128],
                        identity=IDT[:])
                    (nc.vector.tensor_copy if t % 2 == 0
                     else nc.scalar.copy)(
                        out=X16[:, t * 32:(t + 1) * 32], in_=pst[:])

                # u_hat: K=32 matmuls, r2 order cycles the PE row-group;
                # bias added during evac (DVE tensor_scalar / Act Copy)
                w2c = W2C[g]
                for r2 in (0, 2, 4, 6, 1, 3, 5, 7):
                    psA = ups.tile([128, 320], f32, name="upa", tag="upa",
                                   bufs=3)
                    psB = ups.tile([128, 320], f32, name="upb", tag="upb",
                                   bufs=3)
                    for j in range(2):
                        rr = r2 * 2 + j
                        r = g * 16 + rr
                        rq = r // 36
                        q = (rr // 4) * 32
                        rhs = A(X16, q, 32, [(1, 160)])
                        nc.tensor.matmul(
                            out=psA[:, j * 160:(j + 1) * 160],
                            lhsT=A(w2c, q, 32, [(1, 128)], rr * CO),
                            rhs=rhs, start=True, stop=True,
                            tile_position=(q, 0))
                        nc.tensor.matmul(
                            out=A(psB, rq * 32, 32, [(1, 160)], j * 160),
                            lhsT=A(w2c, q, 32, [(1, 32)], rr * CO + 128),
                            rhs=rhs, start=True, stop=True,
                            tile_position=(q, rq * 32))
                    r0 = g * 16 + r2 * 2
                    rq0, rl0 = r0 // 36, r0 % 36
                    outA = A(UH_m, 0, 128, [(1, 320)], r0 * 160)
                    outB = A(UH_t, rq0 * 32, 32, [(1, 320)], rl0 * 160)
                    inB = A(psB, rq0 * 32, 32, [(1, 320)])
                    biasB = A(BIAST, rq0 * 32, 32, [(1, 1)])
                    ei = (g * 8 + (0, 2, 4, 6, 1, 3, 5, 7).index(r2))
                    eA, eB = [(0, 1), (1, 0)][ei % 2]

                    def _evac(e, o, i, bias):
                        if e == 0:
                            nc.vector.tensor_scalar(
                                out=o, in0=i, scalar1=bias,
                                scalar2=None, op0=Alu.add)
                        elif e == 1:
                            nc.scalar.activation(
                                out=o, in_=i, func=Act.Identity, bias=bias)
                        else:
                            nc.gpsimd.tensor_scalar(
                                out=o, in0=i, scalar1=bias,
                                scalar2=None, op0=Alu.add)
                    _evac(eA, outA, psA[:, 0:320], BIASM[:, 0:1])
                    _evac(eB, outB, inB, biasB)

        if stage == "uhat":
            nc.sync.dma_start(out=D(outm_d, [(32, 128), (1, 32)]),
                              in_=A(UH_m, 0, 128, [(1, 32)]).bitcast(f32))
            return
        # =========== Phase B: digit-caps loop (route-sharded) ========
        dpool = stk.enter_context(tc.tile_pool(name="dig", bufs=1))
        dups = stk.enter_context(tc.tile_pool(name="dups", bufs=2))
        dps = stk.enter_context(tc.tile_pool(name="dpsum", bufs=2,
                                             space="PSUM"))

        NM = RL * 32            # 4608
        NTT = 36 * 32           # 1152
        SELA = dpool.tile([128, 160], f32, name="sela", tag="sela")
        SELT4 = dpool.tile([128, 160], f32, name="selt4", tag="selt4")
        nc.sync.dma_start(out=SELA[:], in_=sela_d[:])
        nc.sync.dma_start(out=SELT4[:], in_=selt_d[:])

        # state tiles
        DS_m = [dpool.tile([128, NM], f16, name=f"dsm{i}", tag=f"dsm{i}")
                for i in range(2)]
        DS_t = [dpool.tile([128, NTT], f16, name=f"dst{i}", tag=f"dst{i}")
                for i in range(2)]
        MDN_m = dpool.tile([128, NM], f16, name="mdnm", tag="mdnm")
        MDN_t = dpool.tile([128, NTT], f16, name="mdnt", tag="mdnt")
        TR_m = dpool.tile([128, NM], f16, name="trm", tag="trm")
        TR_t = dpool.tile([128, NTT], f16, name="trt", tag="trt")
        Y_m = dpool.tile([128, NM], f16, name="ym", tag="ym")
        Y_t = dpool.tile([128, NTT], f16, name="yt", tag="yt")
        BIJ_m = dpool.tile([128, RL], f16, name="bijm", tag="bijm")
        BIJ_t = dpool.tile([128, 36], f16, name="bijt", tag="bijt")
        SJP_m = dpool.tile([128, 32], f32, name="sjpm", tag="sjpm")
        SJPq = dpool.tile([128, 32], f32, name="sjpq", tag="sjpq")
        SJQ4 = dpool.tile([32, 128], f32, name="sjq4", tag="sjq4")
        SJP_t = dpool.tile([32, 32], f32, name="sjpt", tag="sjpt")
        SJ16m = dpool.tile([128, 32], f16, name="sj16m", tag="sj16m")
        SJ16t = dpool.tile([32, 32], f16, name="sj16t", tag="sj16t")
        SJF_m = dpool.tile([128, 32], f32, name="sjfm", tag="sjfm")
        SJF_t = dpool.tile([32, 32], f32, name="sjft", tag="sjft")
        A2_m = dpool.tile([128, 32], f32, name="a2m", tag="a2m")
        A2_t = dpool.tile([32, 32], f32, name="a2t", tag="a2t")
        M2_m = dpool.tile([128, 32], f32, name="m2m", tag="m2m")
        M2_t = dpool.tile([32, 32], f32, name="m2t", tag="m2t")
        D2_m = dpool.tile([128, 32], f16, name="d2m", tag="d2m")
        D2_t = dpool.tile([32, 32], f16, name="d2t", tag="d2t")
        D2R = dpool.tile([128, 32], f16, name="d2r", tag="d2r")
        OUT_m = dpool.tile([128, 32], f32, name="outm", tag="outm")
        OUT_t = dpool.tile([32, 32], f32, name="outt", tag="outt")
        ZB_m = dpool.tile([128, RL], f32, name="zbm", tag="zbm")
        ZB_t = dpool.tile([128, 36], f32, name="zbt", tag="zbt")
        DPDF = dpool.tile([128, 36], f32, name="dpdf", tag="dpdf")

        bij0 = float(np.float32(1.0) / np.float32(R))
        nc.vector.memset(BIJ_m[:], bij0)
        nc.vector.memset(BIJ_t[:], bij0)
        NEG1 = dpool.tile([128, 1], f32, name="neg1", tag="neg1")
        ZERO = dpool.tile([128, 1], f32, name="zero", tag="zero")
        nc.vector.memset(NEG1[:], -1.0)
        nc.vector.memset(ZERO[:], 0.0)

        HR = 72                 # r-split point for z/tree engine split

        def Um(t):
            return A(UH_m, 0, 128, [(5 * 32, RL), (1, 32)], t * 32)

        def Ut(t):
            return A(UH_t, 0, 128, [(5 * 32, 36), (1, 32)], t * 32)

        def spikes(t, dsm, dst):
            """DS = (U > 1) on DVE — exact {0,1}."""
            nc.vector.tensor_scalar(out=dsm[:], in0=Um(t), scalar1=1.0,
                                    scalar2=None, op0=Alu.is_gt)
            nc.vector.tensor_scalar(out=dst[:], in0=Ut(t), scalar1=1.0,
                                    scalar2=None, op0=Alu.is_gt)

        # ---- preamble: t=0 spikes / reset / trace ----
        spikes(0, DS_m[0], DS_t[0])
        nc.vector.tensor_tensor(out=MDN_m[:], in0=Um(0), in1=DS_m[0][:],
                                op=Alu.subtract)
        nc.vector.tensor_tensor(out=MDN_t[:], in0=Ut(0), in1=DS_t[0][:],
                                op=Alu.subtract)
        nc.scalar.activation(out=TR_m[:], in_=DS_m[0][:], func=Act.Copy)
        nc.scalar.activation(out=TR_t[:], in_=DS_t[0][:], func=Act.Copy)

        for t in range(T):
            cm, ct = DS_m[t % 2], DS_t[t % 2]
            nm, nt = DS_m[(t + 1) % 2], DS_t[(t + 1) % 2]
            Ymv = A(Y_m, 0, 128, [(32, RL), (1, 32)])
            Ytv = A(Y_t, 0, 128, [(32, 36), (1, 32)])

            # ---- y = ds * bij (Pool) quartered over batch, pipelined
            # with the s_j partial reduces (DVE) so the reduce starts
            # after the first quarter of y instead of all of it ----
            for q in range(4):
                nc.gpsimd.tensor_tensor(
                    out=A(Y_m, 0, 128, [(32, RL), (1, 8)], q * 8),
                    in0=A(cm, 0, 128, [(32, RL), (1, 8)], q * 8),
                    in1=A(BIJ_m, 0, 128, [(1, RL), (0, 8)]),
                    op=Alu.mult)
            nc.gpsimd.tensor_tensor(
                out=Ytv, in0=A(ct, 0, 128, [(32, 36), (1, 32)]),
                in1=A(BIJ_t, 0, 128, [(1, 36), (0, 32)]), op=Alu.mult)
            # trace update for this step (DVE; Pool has no TensorScalarPtr
            # on HW); z of this step reads it after the AllGather
            if 0 < t < T - 1:
                nc.vector.scalar_tensor_tensor(
                    out=TR_m[:], in0=TR_m[:], scalar=float(DECAY_TR),
                    in1=cm[:], op0=Alu.mult, op1=Alu.max)
                nc.vector.scalar_tensor_tensor(
                    out=TR_t[:], in0=TR_t[:], scalar=float(DECAY_TR),
                    in1=ct[:], op0=Alu.mult, op1=Alu.max)
            for q in range(4):
                nc.vector.tensor_reduce(
                    out=A(SJP_m, 0, 128, [(1, 8)], q * 8),
                    in_=A(Y_m, 0, 128, [(1, 8), (32, RL)], q * 8),
                    axis=mybir.AxisListType.X, op=Alu.add)
            nc.vector.tensor_reduce(
                out=SJPq[:], in_=A(Y_t, 0, 128, [(1, 32), (32, 36)]),
                axis=mybir.AxisListType.X, op=Alu.add)
            # fold tail partial [128 (rq,cot), 32] -> [32 cot, (rq 4, 32)]
            for rq in range(4):
                (nc.sync if rq % 2 else nc.gpsimd).dma_start(
                    out=A(SJQ4, 0, 32, [(1, 32)], rq * 32),
                    in_=A(SJPq, rq * 32, 32, [(1, 32)]))
            nc.vector.tensor_reduce(
                out=SJP_t[:], in_=A(SJQ4, 0, 32, [(1, 32), (32, 4)]),
                axis=mybir.AxisListType.X, op=Alu.add)
            # local out_mem accumulation (host sums cores)
            if t == 0:
                nc.vector.tensor_copy(out=OUT_m[:], in_=SJP_m[:])
                nc.vector.tensor_copy(out=OUT_t[:], in_=SJP_t[:])
            else:
                nc.vector.tensor_tensor(out=OUT_m[:], in0=OUT_m[:],
                                        in1=SJP_m[:], op=Alu.add)
                nc.vector.tensor_tensor(out=OUT_t[:], in0=OUT_t[:],
                                        in1=SJP_t[:], op=Alu.add)
            if t == T - 1:
                break

            nc.vector.tensor_copy(out=SJ16m[:], in_=SJP_m[:])
            nc.vector.tensor_copy(out=SJ16t[:], in_=SJP_t[:])
            nc.sync.dma_start(out=D(sj_in[t], [(32, 128), (1, 32)]),
                              in_=SJ16m[:])
            nc.scalar.dma_start(out=D(sj_in[t], [(32, 32), (1, 32)], 4096),
                              in_=SJ16t[:])

            # ---- a-pass for t+1 on DVE (runs during the collective;
            # Pool is blocked while the gpsimd queue owns the AG) ----
            nc.vector.scalar_tensor_tensor(
                out=Um(t + 1), in0=MDN_m[:], scalar=0.2,
                in1=Um(t + 1), op0=Alu.mult, op1=Alu.add)
            nc.vector.scalar_tensor_tensor(
                out=Ut(t + 1), in0=MDN_t[:], scalar=0.2,
                in1=Ut(t + 1), op0=Alu.mult, op1=Alu.add)

            if solo:
                nc.sync.dma_start(
                    out=D(sj_out[t], [(1, SJP)]), in_=sj_in[t][:])
            else:
                nc.gpsimd.collective_compute(
                    "AllGather", Alu.bypass, replica_groups=rg,
                    ins=[sj_in[t][:]], outs=[sj_out[t][:]])

            # ---- during-AG: spikes t+1 on Act ----
            spikes(t + 1, nm, nt)

            # ---- post-AG: gather s_j, dig2 membranes ----
            SJG_m = dups.tile([128, 8 * 32], f16, name="sjgm", tag="sjgm")
            SJG_t = dups.tile([32, 8 * 32], f16, name="sjgt", tag="sjgt")
            nc.sync.dma_start(
                out=A(SJG_m, 0, 128, [(32, 8), (1, 32)]),
                in_=D(sj_out[t], [(32, 128), (SJP, 8), (1, 32)]))
            nc.scalar.dma_start(
                out=A(SJG_t, 0, 32, [(32, 8), (1, 32)]),
                in_=D(sj_out[t], [(32, 32), (SJP, 8), (1, 32)], 4096))
            nc.vector.tensor_reduce(
                out=SJF_m[:], in_=A(SJG_m, 0, 128, [(1, 32), (32, 8)]),
                axis=mybir.AxisListType.X, op=Alu.add)
            nc.vector.tensor_reduce(
                out=SJF_t[:], in_=A(SJG_t, 0, 32, [(1, 32), (32, 8)]),
                axis=mybir.AxisListType.X, op=Alu.add)

            if t == 0:
                a2m, a2t = SJF_m, SJF_t
            else:
                nc.vector.scalar_tensor_tensor(
                    out=A2_m[:], in0=M2_m[:], scalar=0.2, in1=SJF_m[:],
                    op0=Alu.mult, op1=Alu.add)
                nc.vector.scalar_tensor_tensor(
                    out=A2_t[:], in0=M2_t[:], scalar=0.2, in1=SJF_t[:],
                    op0=Alu.mult, op1=Alu.add)
                a2m, a2t = A2_m, A2_t
            nc.vector.tensor_scalar(
                out=D2_m[:], in0=a2m[:], scalar1=0.5, scalar2=None,
                op0=Alu.is_gt)
            nc.vector.tensor_scalar(
                out=D2_t[:], in0=a2t[:], scalar1=0.5, scalar2=None,
                op0=Alu.is_gt)
            for rq in range(4):
                (nc.sync if rq % 2 else nc.scalar).dma_start(
                    out=A(D2R, rq * 32, 32, [(1, 32)]), in_=D2_t[:])
            if t < T - 2:
                nc.vector.scalar_tensor_tensor(
                    out=M2_m[:], in0=D2_m[:], scalar=-0.5, in1=a2m[:],
                    op0=Alu.mult, op1=Alu.add)
                nc.vector.scalar_tensor_tensor(
                    out=M2_t[:], in0=D2_t[:], scalar=-0.5, in1=a2t[:],
                    op0=Alu.mult, op1=Alu.add)

            # ---- z = (trace - 0.1) * d2s (r-split DVE/Pool onto Y) ----
            nc.vector.scalar_tensor_tensor(
                out=A(Y_m, 0, 128, [(32, HR), (1, 32)]),
                in0=A(TR_m, 0, 128, [(32, HR), (1, 32)]),
                scalar=-0.1, in1=A(D2_m, 0, 128, [(0, HR), (1, 32)]),
                op0=Alu.add, op1=Alu.mult)
            nc.vector.scalar_tensor_tensor(
                out=A(Y_m, 0, 128, [(32, RL - HR), (1, 32)], HR * 32),
                in0=A(TR_m, 0, 128, [(32, RL - HR), (1, 32)], HR * 32),
                scalar=-0.1,
                in1=A(D2_m, 0, 128, [(0, RL - HR), (1, 32)]),
                op0=Alu.add, op1=Alu.mult)
            nc.vector.scalar_tensor_tensor(
                out=Ytv, in0=A(TR_t, 0, 128, [(32, 36), (1, 32)]),
                scalar=-0.1, in1=A(D2R, 0, 128, [(0, 36), (1, 32)]),
                op0=Alu.add, op1=Alu.mult)
            # fold b 32->8 (tree), then reduce -> ZB[co, r]
            for w in (16, 8):
                nc.vector.tensor_tensor(
                    out=A(Y_m, 0, 128, [(32, HR), (1, w)]),
                    in0=A(Y_m, 0, 128, [(32, HR), (1, w)]),
                    in1=A(Y_m, 0, 128, [(32, HR), (1, w)], w),
                    op=Alu.add)
                nc.gpsimd.tensor_tensor(
                    out=A(Y_m, 0, 128, [(32, RL - HR), (1, w)], HR * 32),
                    in0=A(Y_m, 0, 128, [(32, RL - HR), (1, w)], HR * 32),
                    in1=A(Y_m, 0, 128, [(32, RL - HR), (1, w)],
                          HR * 32 + w),
                    op=Alu.add)
                nc.gpsimd.tensor_tensor(
                    out=A(Y_t, 0, 128, [(32, 36), (1, w)]),
                    in0=A(Y_t, 0, 128, [(32, 36), (1, w)]),
                    in1=A(Y_t, 0, 128, [(32, 36), (1, w)], w),
                    op=Alu.add)
            nc.vector.tensor_reduce(
                out=A(ZB_m, 0, 128, [(1, HR)]),
                in_=A(Y_m, 0, 128, [(32, HR), (1, 8)]),
                axis=mybir.AxisListType.X, op=Alu.add)
            nc.vector.tensor_reduce(
                out=A(ZB_m, 0, 128, [(1, RL - HR)], HR),
                in_=A(Y_m, 0, 128, [(32, RL - HR), (1, 8)], HR * 32),
                axis=mybir.AxisListType.X, op=Alu.add)
            nc.vector.tensor_reduce(
                out=ZB_t[:], in_=A(Y_t, 0, 128, [(32, 36), (1, 8)]),
                axis=mybir.AxisListType.X, op=Alu.add)
            # ---- delta matmuls: PD[co', r] = sum_co sel * zb; the co
            # tail accumulates per rq-block straight from ZB_t ----
            PD_m = dps.tile([128, 145], f32, name="pdm", tag="pdm")
            PD_t = dps.tile([32, 145], f32, name="pdt", tag="pdt")
            for rq in range(4):
                cs = slice(rq * 36, (rq + 1) * 36)
                nc.tensor.matmul(out=PD_m[:, cs],
                                 lhsT=SELA[:, 0:128], rhs=ZB_m[:, cs],
                                 start=True, stop=False)
                nc.tensor.matmul(
                    out=PD_m[:, cs],
                    lhsT=A(SELT4, rq * 32, 32, [(1, 128)]),
                    rhs=A(ZB_t, rq * 32, 32, [(1, 36)]),
                    start=False, stop=True,
                    tile_position=(rq * 32, 0))
                nc.tensor.matmul(out=PD_t[:, cs],
                                 lhsT=SELA[:, 128:160], rhs=ZB_m[:, cs],
                                 start=True, stop=False)
                nc.tensor.matmul(
                    out=PD_t[:, cs],
                    lhsT=A(SELT4, rq * 32, 32, [(1, 32)], 128),
                    rhs=A(ZB_t, rq * 32, 32, [(1, 36)]),
                    start=False, stop=True,
                    tile_position=(rq * 32, 0))

            # ---- bij updates; tail folded via one PSUM->SBUF DMA ----
            nc.vector.scalar_tensor_tensor(
                out=BIJ_m[:], in0=PD_m[:, 0:144], scalar=float(ALPHA),
                in1=BIJ_m[:], op0=Alu.mult, op1=Alu.add)
            PDTS = dups.tile([32, 145], f32, name="pdts", tag="pdts")
            nc.scalar.copy(out=PDTS[:, 0:144], in_=PD_t[:, 0:144])
            for rq in range(4):
                (nc.sync if rq % 2 else nc.scalar).dma_start(
                    out=A(DPDF, rq * 32, 32, [(1, 36)]),
                    in_=A(PDTS, 0, 32, [(1, 36)], rq * 36))
            nc.vector.scalar_tensor_tensor(
                out=BIJ_t[:], in0=DPDF[:], scalar=float(ALPHA),
                in1=BIJ_t[:], op0=Alu.mult, op1=Alu.add)

            # ---- slack: reset (DVE) for t+1; feeds a-pass t+2 ----
            if t < T - 2:
                nc.vector.tensor_tensor(out=MDN_m[:], in0=Um(t + 1),
                                        in1=nm[:], op=Alu.subtract)
                nc.vector.tensor_tensor(out=MDN_t[:], in0=Ut(t + 1),
                                        in1=nt[:], op=Alu.subtract)

        # ---- write local-partial outputs ----
        nc.sync.dma_start(out=D(outm_d, [(32, 128), (1, 32)]),
                          in_=OUT_m[:])
        nc.sync.dma_start(out=D(outm_d, [(32, 32), (1, 32)], 128 * 32),
                          in_=OUT_t[:])


def _host_prepare(data, conv_w, conv_b, prim_w, prim_b, W, bias):
    """Build per-core input maps."""
    from numpy.lib.stride_tricks import sliding_window_view
    f32 = np.float32
    data = np.asarray(data, f32)
    conv_w = np.asarray(conv_w, f32)
    conv_b = np.asarray(conv_b, f32)
    prim_w = np.asarray(prim_w, f32)
    prim_b = np.asarray(prim_b, f32)
    W = np.asarray(W, f32)
    bias = np.asarray(bias, f32)

    # im2col: win[b, ky, kx, oy, ox]
    win = sliding_window_view(data[:, 0, :, :], (20, 20), axis=(1, 2))
    im2_all = np.ascontiguousarray(win).reshape(B, 81, 400)

    # everything feeding the spiking membranes runs in a 2x-scaled domain
    # (exact in fp32) so the reset is the plain subtract M = A - ds.
    convw = np.ascontiguousarray(conv_w.reshape(256, 81).T) * f32(2.0)
    convb2 = np.ascontiguousarray(conv_b.reshape(2, 128).T) * f32(2.0)

    # prim weights kc-major: [kc, pos, k, mc*128+m]
    pw = prim_w.reshape(2, 128, 2, 128, 9, 9)   # [mc, m, kc, k, ky, kx]
    primw = np.ascontiguousarray(
        pw.transpose(2, 4, 5, 3, 0, 1).reshape(2 * 81 * 128 * 256)) \
        * f32(2.0)
    primb2 = np.ascontiguousarray(prim_b.reshape(2, 128).T) * f32(2.0)

    # W2[i, r, co] with co = o*10 + c, zero-padded to K=32 route-quads:
    # w2g[g, rr*8+i, rr*160+co] = 2*W2[i, g*16+rr, co]  (fp16)
    Wt = np.ascontiguousarray(
        W.transpose(3, 0, 2, 1)).reshape(8, R, CO) * f32(2.0)

    bias_o = bias[:, 0]
    bias2 = np.array([f32(2.0) * bias_o[co // 10] for co in range(CO)], f32)
    biasm = np.ascontiguousarray(bias2[:128].reshape(128, 1))
    biast = np.zeros((128, 1), f32)
    for p in range(128):
        biast[p, 0] = bias2[128 + p % 32]

    cos = np.arange(CO)
    sela = (np.equal.outer(cos[:128] % 10, cos % 10)).astype(f32)
    selt = (np.equal.outer(cos[128:] % 10, cos % 10)).astype(f32)
    sela = np.ascontiguousarray(sela)
    selt = np.ascontiguousarray(np.tile(selt, (4, 1)))

    in_maps = []
    for k in range(N_CORES):
        im2 = np.ascontiguousarray(
            im2_all[BL * k:BL * (k + 1)].transpose(1, 0, 2).reshape(81, 1600))
        w2core = Wt[:, RL * k:RL * (k + 1), :]          # [8, 144, 160]
        w2g = np.zeros((9, 128, 16 * CO), np.float16)
        for rr in range(16):
            blk = w2core[:, rr::16, :]
            w2g[:, rr * 8:(rr + 1) * 8, rr * CO:(rr + 1) * CO] = \
                blk.transpose(1, 0, 2).astype(np.float16)
        in_maps.append({
            "im2": im2, "convw": convw, "convb": convb2,
            "primw": primw, "primb": primb2, "w2g": w2g,
            "biasm": biasm, "biast": biast, "sela": sela, "selt": selt,
        })
    return in_maps


def _postprocess(outs):
    """sum per-core outm [160, 32] partials -> classes [32, 10]."""
    outm = np.zeros((160, 32), np.float32)
    for o in outs:
        outm += np.asarray(o, np.float32)
    out3 = outm.reshape(16, 10, 32) / np.float32(T)
    sq = (out3 * out3).sum(axis=0)
    return np.sqrt(sq).T.astype(np.float32)


def kernel(data, conv_w, conv_b, prim_w, prim_b, W, bias, time_window):
    from concourse.bass_utils import run_bass_kernel_spmd
    assert int(time_window) == T
    if "nc" not in _CACHE:
        _CACHE["nc"] = _build_program()
    nc = _CACHE["nc"]
    in_maps = _host_prepare(data, conv_w, conv_b, prim_w, prim_b, W, bias)
    res = run_bass_kernel_spmd(nc, in_maps, core_ids=list(range(N_CORES)))
    return _postprocess([r["outm"] for r in res.results])
